# revision 1
# baseline (speedup 1.0000x reference)
"""CredLightGCN (3-layer LightGCN propagation + batch dot readout) on 8
Trainium2 NeuronCores.

Strategy (all sizes hardcoded for the nn_CredLightGCN problem):
  * The six SpMMs (2 directions x 3 layers) are computed as PE one-hot
    matmuls: for each destination group of 128 rows, PSUM accumulates
    chunks  out[seg,d] += M[slot,seg]^T @ G[slot,d]  where M is a
    host-precomputed (multi-hot, edge-value-weighted) selection matrix
    streamed from HBM in bf16 and G holds the source rows for the group's
    (deduplicated) edge slots.
  * Layer 1 needs no on-device gathers: G streams from host-expanded edge
    tables (the inputs are known on the host).
  * Layer 2 gathers source rows with gpsimd dma_gather (256B rows, int16
    indices, tables split in 25088-row quarters), which is the
    descriptor-generation-bound core cost of the kernel.
  * Layer 3 is batch-funneled: only rows reachable from the 16384 query
    pairs are produced (batch positions are the destination rows).
  * Tables are bf16 padded to 128 cols (256B rows) so dma_gather's 256B
    element constraint holds; PSUM accumulation stays f32.
  * Cores own disjoint destination-row shards; full tables are rebuilt
    between passes with DRAM AllGather collectives.
  * Readout: per 128 query positions, gather s1/s2 rows as 1KB "quad" rows
    (4 padded rows per descriptor, index = row//4), select the right
    sub-row with a bf16 mask + axis reduce, add the layer-0 and layer-3
    terms, multiply sides and row-reduce.

Row permutation: items/users are assigned to device rows by a
degree-balanced snake so every destination group has a near-equal edge
count, which makes the chunk schedule uniform across the 8 cores (all
cores run one shared program; per-core data differs).
"""

import numpy as np
import ml_dtypes

NCORES = 8
GSZ = 128         # dst rows per group (PSUM partitions)
CH = 128          # edge slots per chunk (PE contraction K)
D = 64            # embedding dim
DP = 128          # padded bf16 row width (256B)
BF = ml_dtypes.bfloat16

N_IT_REAL, N_US_REAL = 50000, 100000
UNIT = NCORES * GSZ
N_IT = -(-N_IT_REAL // UNIT) * UNIT          # 50176
N_US = -(-N_US_REAL // UNIT) * UNIT          # 100352
GI, GU = N_IT // GSZ, N_US // GSZ
GI_C, GU_C = GI // NCORES, GU // NCORES
QS = 25088
NQ_US, NQ_IT = -(-N_US // QS), -(-N_IT // QS)    # 4, 2
BATCH = 16384
BPC = BATCH // NCORES
G3 = BPC // GSZ
RG = 128
NRG = BPC // RG


# --------------------------------------------------------------------------
# host planning
# --------------------------------------------------------------------------

def _balanced_perm(deg, n_pad, n_groups):
    n_real = len(deg)
    order = np.argsort(-deg, kind="stable")
    order = np.concatenate([order, np.arange(n_real, n_pad)])
    gsz = n_pad // n_groups
    pi = np.empty(n_pad, np.int64)
    for r in range(gsz):
        blk = order[r * n_groups:(r + 1) * n_groups]
        cells = np.arange(n_groups) if r % 2 == 0 else \
            np.arange(n_groups - 1, -1, -1)
        pi[blk] = cells * gsz + r
    return pi


def _build_dir_layout(dst_rows, src_rows, vals, groups_per_core, nq, qsize):
    g = dst_rows // GSZ
    seg = (dst_rows % GSZ).astype(np.int32)
    q = src_rows // qsize
    srcl = src_rows % qsize
    core = g // groups_per_core
    gl = g % groups_per_core

    sort_key = np.lexsort((srcl, q, gl, core))
    core_s, gl_s = core[sort_key], gl[sort_key]
    q_s, srcl_s = q[sort_key], srcl[sort_key]
    seg_s, val_s = seg[sort_key], vals[sort_key]
    ck = (core_s * groups_per_core + gl_s) * nq + q_s
    nruns = NCORES * groups_per_core * nq
    new_src = np.empty(len(ck), bool)
    new_src[0] = True
    new_src[1:] = (ck[1:] != ck[:-1]) | (srcl_s[1:] != srcl_s[:-1])
    uniq_cum = np.cumsum(new_src)
    cum0 = np.concatenate([[0], uniq_cum])
    run_start = np.searchsorted(ck, np.arange(nruns))
    uniques_before = cum0[run_start]
    rank = (uniq_cum - 1) - uniques_before[ck]
    ucnt = np.bincount(ck[new_src], minlength=nruns)
    ucnt = ucnt.reshape(NCORES, groups_per_core, nq)

    C = np.maximum(1, -(-ucnt.max(axis=0) // CH))
    sumC = int(C.sum(axis=1).max())
    for i in range(groups_per_core):
        C[i, nq - 1] += sumC - C[i].sum()
    qoff = np.zeros((groups_per_core, nq + 1), np.int64)
    qoff[:, 1:] = np.cumsum(C, axis=1)

    nslots = sumC * CH
    srcs = np.zeros((NCORES, groups_per_core, nslots), np.int32)
    slot = (qoff[gl_s, q_s] * CH + rank).astype(np.int64)
    srcs[core_s[new_src], gl_s[new_src], slot[new_src]] = \
        (q_s[new_src] * qsize + srcl_s[new_src]).astype(np.int32)
    pad = np.ones((NCORES, groups_per_core, nslots), bool)
    pad[core_s[new_src], gl_s[new_src], slot[new_src]] = False
    c_of = np.arange(nslots) // CH
    qof_slot = np.zeros((groups_per_core, nslots), np.int64)
    for i in range(groups_per_core):
        qq = np.searchsorted(qoff[i], c_of, side="right") - 1
        qof_slot[i] = np.minimum(qq, nq - 1) * qsize
    srcs = np.where(pad, qof_slot[None, :, :], srcs)

    return dict(C=C, sumC=sumC, qoff=qoff, src=srcs,
                e_core=core_s, e_gl=gl_s, e_slot=slot, e_seg=seg_s,
                e_val=val_s, nq=nq, qsize=qsize,
                groups_per_core=groups_per_core)


def _layout_arrays(lay):
    gpc, sumC = lay["groups_per_core"], lay["sumC"]
    nslots = sumC * CH
    M = np.zeros((NCORES, gpc, CH, sumC, GSZ), np.float32)
    np.add.at(M, (lay["e_core"], lay["e_gl"], lay["e_slot"] % CH,
                  lay["e_slot"] // CH, lay["e_seg"].astype(np.int64)),
              lay["e_val"])
    locidx = (lay["src"] % lay["qsize"]).astype(np.int16)
    w = locidx.reshape(NCORES, gpc, nslots // 16, 16)
    w = np.swapaxes(w, 2, 3)
    idx = np.tile(w, (1, 1, 8, 1))
    return M.astype(BF), idx


def _expand_E(lay, table_glob):
    gpc, sumC = lay["groups_per_core"], lay["sumC"]
    E = table_glob[lay["src"]]
    E = E.reshape(NCORES, gpc, sumC, CH, D)
    return np.ascontiguousarray(np.swapaxes(E, 2, 3)).astype(BF)


def _make_plan(user_emb, item_emb, edge_vals, edge_u, edge_i, users, items):
    p = {}
    deg_it = np.bincount(edge_i, minlength=N_IT_REAL)
    deg_us = np.bincount(edge_u, minlength=N_US_REAL)
    pi_it = _balanced_perm(deg_it, N_IT, GI)
    pi_us = _balanced_perm(deg_us, N_US, GU)

    t0_us = np.zeros((N_US, D), np.float32)
    t0_us[pi_us[:N_US_REAL]] = user_emb
    t0_it = np.zeros((N_IT, D), np.float32)
    t0_it[pi_it[:N_IT_REAL]] = item_emb
    p["t0_us"], p["t0_it"] = t0_us, t0_it

    dst_it = pi_it[edge_i]
    dst_us = pi_us[edge_u]
    ev = edge_vals.astype(np.float32)
    p["ui"] = _build_dir_layout(dst_it, dst_us, ev, GI_C, NQ_US, QS)
    p["iu"] = _build_dir_layout(dst_us, dst_it, ev, GU_C, NQ_IT, QS)

    def edges_of(ids_batch, by_node_sorted, node_ptr, other_rows, vals):
        cnts = node_ptr[ids_batch + 1] - node_ptr[ids_batch]
        tot = int(cnts.sum())
        pos_rep = np.repeat(np.arange(len(ids_batch)), cnts)
        starts = np.repeat(node_ptr[ids_batch], cnts)
        within = np.arange(tot) - np.repeat(np.cumsum(cnts) - cnts, cnts)
        eidx = by_node_sorted[starts + within]
        return pos_rep.astype(np.int64), other_rows[eidx], vals[eidx]

    o_i = np.argsort(edge_i, kind="stable")
    ptr_i = np.zeros(N_IT_REAL + 1, np.int64)
    ptr_i[1:] = np.cumsum(deg_it)
    o_u = np.argsort(edge_u, kind="stable")
    ptr_u = np.zeros(N_US_REAL + 1, np.int64)
    ptr_u[1:] = np.cumsum(deg_us)

    posA, srcA, valA = edges_of(items, o_i, ptr_i, dst_us, ev)
    posB, srcB, valB = edges_of(users, o_u, ptr_u, dst_it, ev)
    p["l3a"] = _build_dir_layout(posA, srcA, valA, G3, NQ_US, QS)
    p["l3b"] = _build_dir_layout(posB, srcB, valB, G3, NQ_IT, QS)

    p["bu_rows"] = pi_us[users].reshape(NCORES, BPC)
    p["bi_rows"] = pi_it[items].reshape(NCORES, BPC)
    p["e0u_b"] = user_emb[users].reshape(NCORES, BPC, D).astype(np.float32)
    p["e0i_b"] = item_emb[items].reshape(NCORES, BPC, D).astype(np.float32)
    return p


def _pad_table_bf16(t):
    out = np.zeros((t.shape[0], DP), BF)
    out[:, :D] = t.astype(BF)
    return out


def _build_device_arrays(p):
    maps = [dict() for _ in range(NCORES)]
    M_ui, idx_ui = _layout_arrays(p["ui"])
    M_iu, idx_iu = _layout_arrays(p["iu"])
    M_3a, idx_3a = _layout_arrays(p["l3a"])
    M_3b, idx_3b = _layout_arrays(p["l3b"])
    E_ui = _expand_E(p["ui"], p["t0_us"])
    E_iu = _expand_E(p["iu"], p["t0_it"])

    def readout_arrays(rows):
        rg = rows.reshape(NCORES, NRG, RG)
        quad = (rg // 4).astype(np.int16)
        r = (rg % 4).astype(np.int64)
        w = quad.reshape(NCORES, NRG, RG // 16, 16)
        w = np.swapaxes(w, 2, 3)
        idxr = np.tile(w, (1, 1, 8, 1))
        mask = np.zeros((NCORES, NRG, RG, 4 * DP), BF)
        cc = np.arange(NCORES)[:, None, None]
        gg = np.arange(NRG)[None, :, None]
        kk = np.arange(RG)[None, None, :]
        for d in range(D):
            mask[cc, gg, kk, r * DP + d] = 1.0
        return idxr, mask

    idxr_u, mask_u = readout_arrays(p["bu_rows"])
    idxr_i, mask_i = readout_arrays(p["bi_rows"])

    for c in range(NCORES):
        m = maps[c]
        m["M_ui"], m["idx_ui"], m["E_ui"] = M_ui[c], idx_ui[c], E_ui[c]
        m["M_iu"], m["idx_iu"], m["E_iu"] = M_iu[c], idx_iu[c], E_iu[c]
        m["M_3a"], m["idx_3a"] = M_3a[c], idx_3a[c]
        m["M_3b"], m["idx_3b"] = M_3b[c], idx_3b[c]
        m["idxr_u"], m["mask_u"] = idxr_u[c], mask_u[c]
        m["idxr_i"], m["mask_i"] = idxr_i[c], mask_i[c]
        m["e0su"] = p["e0u_b"][c].reshape(NRG, RG, D)
        m["e0si"] = p["e0i_b"][c].reshape(NRG, RG, D)
    return maps


# --------------------------------------------------------------------------
# bass program
# --------------------------------------------------------------------------

def _build_bass(p):
    import concourse.bacc as bacc
    import concourse.tile as tile
    import concourse.mybir as mybir

    f32, i16, bf16 = mybir.dt.float32, mybir.dt.int16, mybir.dt.bfloat16
    nc = bacc.Bacc("TRN2", target_bir_lowering=False, debug=False,
                   num_devices=NCORES)

    def din(name, shape, dt=bf16):
        return nc.dram_tensor(name, list(shape), dt, kind="ExternalInput")

    lays = {}
    for nm, lay, with_e in [("ui", p["ui"], True), ("iu", p["iu"], True),
                            ("3a", p["l3a"], False), ("3b", p["l3b"], False)]:
        gpc, sumC = lay["groups_per_core"], lay["sumC"]
        t = dict(lay=lay, gpc=gpc, sumC=sumC)
        t["M"] = din(f"M_{nm}", [gpc, CH, sumC, GSZ])
        t["idx"] = din(f"idx_{nm}", [gpc, CH, sumC * CH // 16], i16)
        if with_e:
            t["E"] = din(f"E_{nm}", [gpc, CH, sumC, D])
        lays[nm] = t
    idxr_u = din("idxr_u", [NRG, 128, RG // 16], i16)
    idxr_i = din("idxr_i", [NRG, 128, RG // 16], i16)
    mask_u = din("mask_u", [NRG, RG, 4 * DP])
    mask_i = din("mask_i", [NRG, RG, 4 * DP])
    e0su = din("e0su", [NRG, RG, D], f32)
    e0si = din("e0si", [NRG, RG, D], f32)
    y_out = nc.dram_tensor("y", [BPC], f32, kind="ExternalOutput")

    reps = [list(range(NCORES))]

    with tile.TileContext(nc) as tc:
        with (
            tc.tile_pool(name="mt", bufs=3) as mtp,
            tc.tile_pool(name="gt", bufs=3) as gtp,
            tc.tile_pool(name="ixt", bufs=4) as ixp,
            tc.tile_pool(name="ps", bufs=8, space="PSUM") as psp,
            tc.tile_pool(name="ev", bufs=4) as evp,
            tc.tile_pool(name="ro", bufs=4) as rop,
            tc.tile_pool(name="s3", bufs=1) as s3p,
            tc.tile_pool(name="dram", bufs=1, space="DRAM") as drp,
        ):
            sh = {
                "s1_i": drp.tile([GI_C * GSZ, DP], bf16, name="s1_i_sh"),
                "s1_u": drp.tile([GU_C * GSZ, DP], bf16, name="s1_u_sh"),
                "s2_i": drp.tile([GI_C * GSZ, DP], bf16, name="s2_i_sh"),
                "s2_u": drp.tile([GU_C * GSZ, DP], bf16, name="s2_u_sh"),
            }
            fl = {
                "s1_i": drp.tile([N_IT, DP], bf16, name="s1_i_f"),
                "s1_u": drp.tile([N_US, DP], bf16, name="s1_u_f"),
                "s2_i": drp.tile([N_IT, DP], bf16, name="s2_i_f"),
                "s2_u": drp.tile([N_US, DP], bf16, name="s2_u_f"),
            }
            s3i_sb = s3p.tile([128, G3, D], mybir.dt.float32, name="s3i_sb")
            s3u_sb = s3p.tile([128, G3, D], mybir.dt.float32, name="s3u_sb")

            ev_tiles = []
            for j in range(4):
                t_ = evp.tile([GSZ, DP], bf16, name=f"evst{j}", tag=f"evst{j}")
                nc.vector.memset(t_[:], 0.0)
                ev_tiles.append(t_)

            def run_pass(t, src_tab, n_src, nq, dst_shard, dst_s3):
                lay, gpc, sumC = t["lay"], t["gpc"], t["sumC"]
                C, qoff = lay["C"], lay["qoff"]
                stream = src_tab is None
                for g in range(gpc):
                    mt = mtp.tile([CH, sumC, GSZ], bf16, name="mt", tag="mt")
                    nc.sync.dma_start(mt[:], t["M"].ap()[g])
                    if stream:
                        gt = gtp.tile([CH, sumC, D], bf16, name="gts",
                                      tag="gts")
                        nc.sync.dma_start(gt[:], t["E"].ap()[g])
                        rhs = lambda c: gt[:, c, :]
                    else:
                        gt = gtp.tile([CH, sumC, DP], bf16, name="gtg",
                                      tag="gtg")
                        ixt = ixp.tile([CH, sumC * CH // 16], i16,
                                       name="ixt", tag="ixt")
                        nc.sync.dma_start(ixt[:], t["idx"].ap()[g])
                        for q in range(nq):
                            cq, off = int(C[g, q]), int(qoff[g, q])
                            if cq == 0:
                                continue
                            qlo = q * QS
                            qhi = min((q + 1) * QS, n_src)
                            nc.gpsimd.dma_gather(
                                gt[:, off:off + cq, :],
                                src_tab.opt()[qlo:qhi],
                                ixt[:, off * 8:(off + cq) * 8],
                                cq * CH, cq * CH, DP,
                                single_packet=False,
                            )
                        rhs = lambda c: gt[:, c, 0:D]
                    ps = psp.tile([GSZ, D], mybir.dt.float32, name="ps",
                                  tag="ps", space="PSUM")
                    for cx in range(sumC):
                        nc.tensor.matmul(ps[:], lhsT=mt[:, cx, :],
                                         rhs=rhs(cx), start=(cx == 0),
                                         stop=(cx == sumC - 1))
                    if dst_s3 is None:
                        ev = ev_tiles[g % 4]
                        nc.scalar.copy(ev[:, 0:D], ps[:])
                        nc.sync.dma_start(
                            dst_shard.opt()[g * GSZ:(g + 1) * GSZ, :], ev[:])
                    else:
                        nc.scalar.copy(dst_s3[:, g, :], ps[:])

            def ag(shard, full):
                nc.gpsimd.collective_compute(
                    "AllGather", mybir.AluOpType.bypass, replica_groups=reps,
                    ins=[shard.opt()], outs=[full.opt()])

            run_pass(lays["iu"], None, 0, 0, sh["s1_u"], None)
            ag(sh["s1_u"], fl["s1_u"])
            run_pass(lays["ui"], None, 0, 0, sh["s1_i"], None)
            ag(sh["s1_i"], fl["s1_i"])
            run_pass(lays["ui"], fl["s1_u"], N_US, NQ_US, sh["s2_i"], None)
            ag(sh["s2_i"], fl["s2_i"])
            run_pass(lays["iu"], fl["s1_i"], N_IT, NQ_IT, sh["s2_u"], None)
            ag(sh["s2_u"], fl["s2_u"])
            run_pass(lays["3a"], fl["s2_u"], N_US, NQ_US, None, s3i_sb)
            run_pass(lays["3b"], fl["s2_i"], N_IT, NQ_IT, None, s3u_sb)

            qv = {k: fl[k].opt().rearrange("(n r) d -> n (r d)", r=4)
                  for k in fl}

            def side(rg, idxr, maskt, qv1, qv2, e0t, s3sb):
                ixr = rop.tile([128, RG // 16], i16, name="ixr", tag="ixr")
                nc.sync.dma_start(ixr[:], idxr.ap()[rg])
                mk = rop.tile([RG, 4 * DP], bf16, name="mk", tag="mk")
                nc.sync.dma_start(mk[:], maskt.ap()[rg])
                e0 = rop.tile([RG, D], mybir.dt.float32, name="e0", tag="e0")
                nc.sync.dma_start(e0[:], e0t.ap()[rg])
                acc = rop.tile([RG, D], mybir.dt.float32, name="acc",
                               tag="acc")
                nc.vector.tensor_add(out=acc[:], in0=e0[:],
                                     in1=s3sb[:, rg, :])
                for qvx in (qv1, qv2):
                    gq = rop.tile([RG, 1, 4 * DP], bf16, name="gq", tag="gq")
                    nc.gpsimd.dma_gather(gq[:], qvx, ixr[:], RG, RG, 4 * DP,
                                         single_packet=False)
                    sel = rop.tile([RG, 4 * DP], mybir.dt.float32,
                                   name="sel", tag="sel")
                    nc.vector.tensor_mul(out=sel[:], in0=gq[:, 0, :],
                                         in1=mk[:])
                    red = rop.tile([RG, D], mybir.dt.float32, name="red",
                                   tag="red")
                    nc.vector.reduce_sum(
                        red[:],
                        sel[:].rearrange("p (r d) -> p d r", r=4)[:, 0:D, :],
                        axis=mybir.AxisListType.X)
                    nc.vector.tensor_add(out=acc[:], in0=acc[:], in1=red[:])
                return acc

            yv = y_out.ap().rearrange("(g p) -> g p", p=RG)
            for rg in range(NRG):
                su = side(rg, idxr_u, mask_u, qv["s1_u"], qv["s2_u"], e0su,
                          s3u_sb)
                si = side(rg, idxr_i, mask_i, qv["s1_i"], qv["s2_i"], e0si,
                          s3i_sb)
                pr = rop.tile([RG, D], mybir.dt.float32, name="pr", tag="pr")
                nc.vector.tensor_mul(out=pr[:], in0=su[:], in1=si[:])
                nc.vector.tensor_scalar_mul(out=pr[:], in0=pr[:],
                                            scalar1=1.0 / 16.0)
                yc = rop.tile([RG, 1], mybir.dt.float32, name="yc", tag="yc")
                nc.vector.reduce_sum(yc[:], pr[:], axis=mybir.AxisListType.X)
                nc.sync.dma_start(yv[rg], yc[:, 0])

    nc.compile()
    return nc


_CACHE = {}
_TRACE = False        # set True (by a test harness) to capture an NTFF trace
_TRACE_DIR = None
_LAST_RES = None      # BassKernelResults of the most recent run


def _schedule_key(p):
    import hashlib
    h = hashlib.sha1()
    for k in ("ui", "iu", "l3a", "l3b"):
        h.update(p[k]["C"].tobytes())
        h.update(np.int64(p[k]["sumC"]).tobytes())
    return h.hexdigest()


def kernel(user_emb, item_emb, edge_vals, edge_u, edge_i, users, items):
    global _LAST_RES
    from concourse.bass_utils import run_bass_kernel_spmd

    user_emb = np.asarray(user_emb, np.float32)
    item_emb = np.asarray(item_emb, np.float32)
    edge_vals = np.asarray(edge_vals, np.float32)
    edge_u = np.asarray(edge_u, np.int64)
    edge_i = np.asarray(edge_i, np.int64)
    users = np.asarray(users, np.int64)
    items = np.asarray(items, np.int64)

    p = _make_plan(user_emb, item_emb, edge_vals, edge_u, edge_i, users,
                   items)
    maps = _build_device_arrays(p)
    key = _schedule_key(p)
    if _CACHE.get("key") != key:
        _CACHE["nc"] = _build_bass(p)
        _CACHE["key"] = key
    nc = _CACHE["nc"]
    res = run_bass_kernel_spmd(nc, maps, core_ids=list(range(NCORES)),
                               trace=_TRACE, tmpdir=_TRACE_DIR)
    _LAST_RES = res
    y = np.concatenate([res.results[c]["y"] for c in range(NCORES)])
    return y.astype(np.float32)



# revision 20
# speedup vs baseline: 1.4834x; 1.4834x over previous
"""CredLightGCN (3-layer LightGCN propagation + batch dot readout) on 8
Trainium2 NeuronCores.

Strategy (all sizes hardcoded for the nn_CredLightGCN problem):
  * The six SpMMs (2 directions x 3 layers) are computed as PE one-hot
    matmuls: for each destination group of 128 rows, PSUM accumulates
    chunks  out[seg,d] += OH[slot,seg]^T @ G[slot,d]  where OH is a
    one-hot (edge-value-weighted) selection matrix generated ON DEVICE by
    the vector engines:  OH[p, f] = (iota[f] == seg[p]) * val[p]  via a
    fused tensor_scalar(is_equal, mult) with per-partition scalars.  Only
    the tiny seg/val streams (4B/edge) come from HBM, not the 32KB/chunk
    dense M tiles.  One-hot generation alternates DVE / Pool to balance
    engine load.  One slot per edge (no dedup).
  * Layer 1 needs no on-device gathers: G streams from host-expanded edge
    tables (the inputs are known on the host).
  * Layer 2 gathers source rows with gpsimd dma_gather (256B rows, int16
    indices, tables split in 25088-row windows).
  * Layer 3 is batch-funneled: only rows reachable from the 16384 query
    pairs are produced (batch positions are the destination rows).
  * Tables are bf16 padded to 128 cols (256B rows) to satisfy dma_gather's
    256B element constraint; PSUM accumulation stays f32.
  * Cores own disjoint destination-row shards; full tables are rebuilt
    between passes with DRAM AllGather collectives, overlapped with the
    next pass's compute (l3b is scheduled before l3a so the final
    AllGather hides under it).
  * Readout: gather s1/s2 rows as 1KB "quad" rows (4 padded rows per
    descriptor, index = row//4) in ONE dma_gather per (side, table),
    select the right sub-row with a bf16 mask + axis reduce, add the
    layer-0 and layer-3 terms, multiply sides and row-reduce.

Row permutation: items/users are assigned to device rows by a
degree-balanced snake so every destination group has a near-equal edge
count, which makes the chunk schedule uniform across the 8 cores (all
cores run one shared program; per-core data differs).
"""

import numpy as np
import ml_dtypes

NCORES = 8
GSZ = 128         # dst rows per group (PSUM partitions)
CH = 128          # edge slots per chunk (PE contraction K)
D = 64            # embedding dim
DP = 128          # padded bf16 row width (256B)
BF = ml_dtypes.bfloat16

N_IT_REAL, N_US_REAL = 50000, 100000
UNIT = NCORES * GSZ
N_IT = -(-N_IT_REAL // UNIT) * UNIT          # 50176
N_US = -(-N_US_REAL // UNIT) * UNIT          # 100352
GI, GU = N_IT // GSZ, N_US // GSZ
GI_C, GU_C = GI // NCORES, GU // NCORES
QS = 25088
NQ_US, NQ_IT = -(-N_US // QS), -(-N_IT // QS)    # 4, 2
BATCH = 16384
BPC = BATCH // NCORES
G3 = BPC // GSZ
RG = 128
NRG = BPC // RG


# --------------------------------------------------------------------------
# host planning
# --------------------------------------------------------------------------

def _balanced_perm(deg, n_pad, n_groups):
    n_real = len(deg)
    order = np.argsort(-deg, kind="stable")
    order = np.concatenate([order, np.arange(n_real, n_pad)])
    gsz = n_pad // n_groups
    pi = np.empty(n_pad, np.int64)
    for r in range(gsz):
        blk = order[r * n_groups:(r + 1) * n_groups]
        cells = np.arange(n_groups) if r % 2 == 0 else \
            np.arange(n_groups - 1, -1, -1)
        pi[blk] = cells * gsz + r
    return pi


def _build_dir_layout(dst_rows, src_rows, vals, groups_per_core, nq, qsize):
    """One slot per edge, sorted (core, group, q, src) for gather locality."""
    g = dst_rows // GSZ
    seg = (dst_rows % GSZ).astype(np.int32)
    q = src_rows // qsize
    srcl = src_rows % qsize
    core = g // groups_per_core
    gl = g % groups_per_core

    sort_key = np.lexsort((srcl, q, gl, core))
    core_s, gl_s = core[sort_key], gl[sort_key]
    q_s, srcl_s = q[sort_key], srcl[sort_key]
    seg_s, val_s = seg[sort_key], vals[sort_key]
    ck = (core_s * groups_per_core + gl_s) * nq + q_s
    nruns = NCORES * groups_per_core * nq
    run_start = np.searchsorted(ck, np.arange(nruns + 1))
    cnt = (run_start[1:] - run_start[:-1]).reshape(
        NCORES, groups_per_core, nq)
    rank = np.arange(len(ck)) - run_start[ck]

    C = np.maximum(1, -(-cnt.max(axis=0) // CH))
    sumC = int(C.sum(axis=1).max())
    for i in range(groups_per_core):
        C[i, nq - 1] += sumC - C[i].sum()
    qoff = np.zeros((groups_per_core, nq + 1), np.int64)
    qoff[:, 1:] = np.cumsum(C, axis=1)

    nslots = sumC * CH
    slot = (qoff[gl_s, q_s] * CH + rank).astype(np.int64)

    srcs = np.zeros((NCORES, groups_per_core, nslots), np.int32)
    srcs[core_s, gl_s, slot] = (q_s * qsize + srcl_s).astype(np.int32)
    pad = np.ones((NCORES, groups_per_core, nslots), bool)
    pad[core_s, gl_s, slot] = False
    c_of = np.arange(nslots) // CH
    qof_slot = np.zeros((groups_per_core, nslots), np.int64)
    for i in range(groups_per_core):
        qq = np.searchsorted(qoff[i], c_of, side="right") - 1
        qof_slot[i] = np.minimum(qq, nq - 1) * qsize
    srcs = np.where(pad, qof_slot[None, :, :], srcs)

    return dict(C=C, sumC=sumC, qoff=qoff, src=srcs,
                e_core=core_s, e_gl=gl_s, e_slot=slot, e_seg=seg_s,
                e_val=val_s, nq=nq, qsize=qsize,
                groups_per_core=groups_per_core)


def _layout_arrays(lay):
    """seg||val stream [NC, gpc, CH, 2*sumC] bf16 and wrapped idx tables."""
    gpc, sumC = lay["groups_per_core"], lay["sumC"]
    nslots = sumC * CH
    segval = np.zeros((NCORES, gpc, CH, 2 * sumC), np.float32)
    p = lay["e_slot"] % CH
    cx = lay["e_slot"] // CH
    segval[lay["e_core"], lay["e_gl"], p, cx] = \
        lay["e_seg"].astype(np.float32)
    segval[lay["e_core"], lay["e_gl"], p, sumC + cx] = lay["e_val"]
    locidx = (lay["src"] % lay["qsize"]).astype(np.int16)
    w = locidx.reshape(NCORES, gpc, nslots // 16, 16)
    w = np.swapaxes(w, 2, 3)
    idx = np.tile(w, (1, 1, 8, 1))
    return segval, idx


def _expand_E(lay, table_glob):
    gpc, sumC = lay["groups_per_core"], lay["sumC"]
    E = table_glob[lay["src"]]
    E = E.reshape(NCORES, gpc, sumC, CH, D)
    return np.ascontiguousarray(np.swapaxes(E, 2, 3)).astype(BF)


def _make_plan(user_emb, item_emb, edge_vals, edge_u, edge_i, users, items):
    p = {}
    deg_it = np.bincount(edge_i, minlength=N_IT_REAL)
    deg_us = np.bincount(edge_u, minlength=N_US_REAL)
    pi_it = _balanced_perm(deg_it, N_IT, GI)
    pi_us = _balanced_perm(deg_us, N_US, GU)

    t0_us = np.zeros((N_US, D), np.float32)
    t0_us[pi_us[:N_US_REAL]] = user_emb
    t0_it = np.zeros((N_IT, D), np.float32)
    t0_it[pi_it[:N_IT_REAL]] = item_emb
    p["t0_us"], p["t0_it"] = t0_us, t0_it

    dst_it = pi_it[edge_i]
    dst_us = pi_us[edge_u]
    ev = edge_vals.astype(np.float32)
    p["ui"] = _build_dir_layout(dst_it, dst_us, ev, GI_C, NQ_US, QS)
    p["iu"] = _build_dir_layout(dst_us, dst_it, ev, GU_C, NQ_IT, QS)

    def edges_of(ids_batch, by_node_sorted, node_ptr, other_rows, vals):
        cnts = node_ptr[ids_batch + 1] - node_ptr[ids_batch]
        tot = int(cnts.sum())
        pos_rep = np.repeat(np.arange(len(ids_batch)), cnts)
        starts = np.repeat(node_ptr[ids_batch], cnts)
        within = np.arange(tot) - np.repeat(np.cumsum(cnts) - cnts, cnts)
        eidx = by_node_sorted[starts + within]
        return pos_rep.astype(np.int64), other_rows[eidx], vals[eidx]

    o_i = np.argsort(edge_i, kind="stable")
    ptr_i = np.zeros(N_IT_REAL + 1, np.int64)
    ptr_i[1:] = np.cumsum(deg_it)
    o_u = np.argsort(edge_u, kind="stable")
    ptr_u = np.zeros(N_US_REAL + 1, np.int64)
    ptr_u[1:] = np.cumsum(deg_us)

    posA, srcA, valA = edges_of(items, o_i, ptr_i, dst_us, ev)
    posB, srcB, valB = edges_of(users, o_u, ptr_u, dst_it, ev)
    p["l3a"] = _build_dir_layout(posA, srcA, valA, G3, NQ_US, QS)
    p["l3b"] = _build_dir_layout(posB, srcB, valB, G3, NQ_IT, QS)

    p["bu_rows"] = pi_us[users].reshape(NCORES, BPC)
    p["bi_rows"] = pi_it[items].reshape(NCORES, BPC)
    p["e0u_b"] = user_emb[users].reshape(NCORES, BPC, D).astype(np.float32)
    p["e0i_b"] = item_emb[items].reshape(NCORES, BPC, D).astype(np.float32)
    return p


def _build_device_arrays(p):
    maps = [dict() for _ in range(NCORES)]
    sv_ui, idx_ui = _layout_arrays(p["ui"])
    sv_iu, idx_iu = _layout_arrays(p["iu"])
    sv_3a, idx_3a = _layout_arrays(p["l3a"])
    sv_3b, idx_3b = _layout_arrays(p["l3b"])
    E_ui = _expand_E(p["ui"], p["t0_us"])
    E_iu = _expand_E(p["iu"], p["t0_it"])

    def readout_arrays(rows):
        # one batched gather per table: 2048 quad indices, wrapped 16-wide
        quad = (rows // 4).astype(np.int16)             # [NC, BPC]
        r = (rows.reshape(NCORES, NRG, RG) % 4).astype(np.int64)
        w = quad.reshape(NCORES, BPC // 16, 16)
        w = np.swapaxes(w, 1, 2)                        # [NC, 16, BPC//16]
        idxr = np.tile(w, (1, 8, 1))                    # [NC, 128, BPC//16]
        mask = np.zeros((NCORES, NRG, RG, 4 * DP), BF)
        cc = np.arange(NCORES)[:, None, None]
        gg = np.arange(NRG)[None, :, None]
        kk = np.arange(RG)[None, None, :]
        for d in range(D):
            mask[cc, gg, kk, r * DP + d] = 1.0
        return idxr, mask

    idxr_u, mask_u = readout_arrays(p["bu_rows"])
    idxr_i, mask_i = readout_arrays(p["bi_rows"])

    for c in range(NCORES):
        m = maps[c]
        m["sv_ui"], m["idx_ui"], m["E_ui"] = sv_ui[c], idx_ui[c], E_ui[c]
        m["sv_iu"], m["idx_iu"], m["E_iu"] = sv_iu[c], idx_iu[c], E_iu[c]
        m["sv_3a"], m["idx_3a"] = sv_3a[c], idx_3a[c]
        m["sv_3b"], m["idx_3b"] = sv_3b[c], idx_3b[c]
        m["idxr_u"], m["mask_u"] = idxr_u[c], mask_u[c]
        m["idxr_i"], m["mask_i"] = idxr_i[c], mask_i[c]
        m["e0su"] = p["e0u_b"][c].reshape(NRG, RG, D)
        m["e0si"] = p["e0i_b"][c].reshape(NRG, RG, D)
    return maps


# --------------------------------------------------------------------------
# bass program
# --------------------------------------------------------------------------

def _build_bass(p):
    import concourse.bacc as bacc
    import concourse.tile as tile
    import concourse.mybir as mybir

    from concourse.tile import add_dep_helper

    f32, i16, bf16 = mybir.dt.float32, mybir.dt.int16, mybir.dt.bfloat16
    EQ, MUL = mybir.AluOpType.is_equal, mybir.AluOpType.mult
    nc = bacc.Bacc("TRN2", target_bir_lowering=False, debug=False,
                   num_devices=NCORES)

    def din(name, shape, dt=bf16):
        return nc.dram_tensor(name, list(shape), dt, kind="ExternalInput")

    lays = {}
    for nm, lay, with_e in [("ui", p["ui"], True), ("iu", p["iu"], True),
                            ("3a", p["l3a"], False), ("3b", p["l3b"], False)]:
        gpc, sumC = lay["groups_per_core"], lay["sumC"]
        t = dict(lay=lay, gpc=gpc, sumC=sumC)
        t["sv"] = din(f"sv_{nm}", [gpc, CH, 2 * sumC], f32)
        t["idx"] = din(f"idx_{nm}", [gpc, CH, sumC * CH // 16], i16)
        if with_e:
            t["E"] = din(f"E_{nm}", [gpc, CH, sumC, D])
        lays[nm] = t
    idxr_u = din("idxr_u", [128, BPC // 16], i16)
    idxr_i = din("idxr_i", [128, BPC // 16], i16)
    mask_u = din("mask_u", [NRG, RG, 4 * DP])
    mask_i = din("mask_i", [NRG, RG, 4 * DP])
    e0su = din("e0su", [NRG, RG, D], f32)
    e0si = din("e0si", [NRG, RG, D], f32)
    y_out = nc.dram_tensor("y", [BPC], f32, kind="ExternalOutput")

    reps = [list(range(NCORES))]

    with tile.TileContext(nc) as tc:
        with (
            tc.tile_pool(name="svt", bufs=3) as svp,
            tc.tile_pool(name="oht", bufs=8) as ohp,
            tc.tile_pool(name="gt", bufs=3) as gtp,
            tc.tile_pool(name="ixt", bufs=4) as ixp,
            tc.tile_pool(name="ps", bufs=8, space="PSUM") as psp,
            tc.tile_pool(name="ev", bufs=4) as evp,
            tc.tile_pool(name="ro", bufs=4) as rop,
            tc.tile_pool(name="roq", bufs=1) as roqp,
            tc.tile_pool(name="s3", bufs=1) as s3p,
            tc.tile_pool(name="cst", bufs=1) as cstp,
            tc.tile_pool(name="dram", bufs=1, space="DRAM") as drp,
        ):
            sh = {
                "s1_i": drp.tile([GI_C * GSZ, DP], bf16, name="s1_i_sh"),
                "s1_u": drp.tile([GU_C * GSZ, DP], bf16, name="s1_u_sh"),
                "s2_i": drp.tile([GI_C * GSZ, DP], bf16, name="s2_i_sh"),
                "s2_u": drp.tile([GU_C * GSZ, DP], bf16, name="s2_u_sh"),
            }
            fl = {
                "s1_i": drp.tile([N_IT, DP], bf16, name="s1_i_f"),
                "s1_u": drp.tile([N_US, DP], bf16, name="s1_u_f"),
                "s2_i": drp.tile([N_IT, DP], bf16, name="s2_i_f"),
                "s2_u": drp.tile([N_US, DP], bf16, name="s2_u_f"),
            }
            s3i_sb = s3p.tile([128, G3, D], mybir.dt.float32, name="s3i_sb")
            s3u_sb = s3p.tile([128, G3, D], mybir.dt.float32, name="s3u_sb")

            iota_t = cstp.tile([128, 128], bf16, name="iota_t")
            nc.gpsimd.iota(iota_t[:], pattern=[[1, 128]], base=0,
                           channel_multiplier=0,
                           allow_small_or_imprecise_dtypes=True)

            ev_tiles = []
            for j in range(4):
                t_ = evp.tile([GSZ, DP], bf16, name=f"evst{j}", tag=f"evst{j}")
                nc.vector.memset(t_[:], 0.0)
                ev_tiles.append(t_)

            anchor = [None]   # last Pool instruction of the previous pass

            def run_pass(t, src_tab, n_src, nq, dst_shard, dst_s3):
                lay, gpc, sumC = t["lay"], t["gpc"], t["sumC"]
                C, qoff = lay["C"], lay["qoff"]
                stream = src_tab is None
                prev_anchor, last_pool = anchor[0], None
                for g in range(gpc):
                    sv = svp.tile([CH, 2 * sumC], mybir.dt.float32,
                                  name="sv", tag="sv")
                    nc.sync.dma_start(sv[:], t["sv"].ap()[g])
                    if stream:
                        gt = gtp.tile([CH, sumC, D], bf16, name="gts",
                                      tag="gts")
                        nc.sync.dma_start(gt[:], t["E"].ap()[g])
                        rhs = lambda c: gt[:, c, :]
                    else:
                        gt = gtp.tile([CH, sumC, DP], bf16, name="gtg",
                                      tag="gtg")
                        ixt = ixp.tile([CH, sumC * CH // 16], i16,
                                       name="ixt", tag="ixt")
                        nc.sync.dma_start(ixt[:], t["idx"].ap()[g])
                        for q in range(nq):
                            cq, off = int(C[g, q]), int(qoff[g, q])
                            if cq == 0:
                                continue
                            qlo = q * QS
                            qhi = min((q + 1) * QS, n_src)
                            gi = nc.gpsimd.dma_gather(
                                gt[:, off:off + cq, :],
                                src_tab.opt()[qlo:qhi],
                                ixt[:, off * 8:(off + cq) * 8],
                                cq * CH, cq * CH, DP,
                                single_packet=False,
                            )
                            # Pin early gathers behind the previous pass's
                            # Pool tail: a hoisted gather parks on Pool.SEQ
                            # waiting for the AllGather and head-of-line
                            # blocks every later Pool instruction.
                            if g < 4 and prev_anchor is not None:
                                add_dep_helper(gi.ins, prev_anchor.ins,
                                               reason="pool queue order")
                            last_pool = gi
                        rhs = lambda c: gt[:, c, 0:D]
                    ps = psp.tile([GSZ, D], mybir.dt.float32, name="ps",
                                  tag="ps", space="PSUM")
                    for cx in range(sumC):
                        oh = ohp.tile([CH, GSZ], bf16, name="oh", tag="oh")
                        # Pool takes a quarter of the stream-pass one-hots
                        # (it is ~3x slower per op and busy with gathers in
                        # gather passes).
                        eng = nc.gpsimd if (stream and cx % 3 == 2) \
                            else nc.vector
                        ts_i = eng.tensor_scalar(
                            out=oh[:], in0=iota_t[:],
                            scalar1=sv[:, cx:cx + 1],
                            scalar2=sv[:, sumC + cx:sumC + cx + 1],
                            op0=EQ, op1=MUL)
                        if eng is nc.gpsimd:
                            last_pool = ts_i
                        nc.tensor.matmul(ps[:], lhsT=oh[:],
                                         rhs=rhs(cx), start=(cx == 0),
                                         stop=(cx == sumC - 1))
                    if dst_s3 is None:
                        ev = ev_tiles[g % 4]
                        nc.scalar.copy(ev[:, 0:D], ps[:])
                        nc.sync.dma_start(
                            dst_shard.opt()[g * GSZ:(g + 1) * GSZ, :], ev[:])
                    else:
                        nc.scalar.copy(dst_s3[:, g, :], ps[:])
                if last_pool is not None:
                    anchor[0] = last_pool

            def ag(shard, full):
                nc.gpsimd.collective_compute(
                    "AllGather", mybir.AluOpType.bypass, replica_groups=reps,
                    ins=[shard.opt()], outs=[full.opt()])

            # Every AllGather overlaps the immediately following pass:
            #   AG(s1_i) under iu-L1, AG(s1_u) under iu-L2 (consumes s1_i),
            #   AG(s2_u) under ui-L2 (consumes s1_u), AG(s2_i) under l3a
            #   (consumes s2_u); l3b consumes s2_i.
            run_pass(lays["ui"], None, 0, 0, sh["s1_i"], None)
            ag(sh["s1_i"], fl["s1_i"])
            run_pass(lays["iu"], None, 0, 0, sh["s1_u"], None)
            ag(sh["s1_u"], fl["s1_u"])
            run_pass(lays["iu"], fl["s1_i"], N_IT, NQ_IT, sh["s2_u"], None)
            ag(sh["s2_u"], fl["s2_u"])
            run_pass(lays["ui"], fl["s1_u"], N_US, NQ_US, sh["s2_i"], None)
            ag(sh["s2_i"], fl["s2_i"])
            run_pass(lays["3a"], fl["s2_u"], N_US, NQ_US, None, s3i_sb)
            run_pass(lays["3b"], fl["s2_i"], N_IT, NQ_IT, None, s3u_sb)

            qv = {k: fl[k].opt().rearrange("(n r) d -> n (r d)", r=4)
                  for k in fl}

            # batched readout quad gathers: one call per (side, table)
            gq = {}
            for nm, idxr in (("s1_u", idxr_u), ("s2_u", idxr_u),
                             ("s1_i", idxr_i), ("s2_i", idxr_i)):
                ixr = roqp.tile([128, BPC // 16], i16, name=f"ixr_{nm}")
                nc.sync.dma_start(ixr[:], idxr.ap())
                gq[nm] = roqp.tile([RG, NRG, 4 * DP], bf16, name=f"gq_{nm}")
                gi = nc.gpsimd.dma_gather(gq[nm][:], qv[nm], ixr[:], BPC,
                                          BPC, 4 * DP, single_packet=False)
                if anchor[0] is not None:
                    add_dep_helper(gi.ins, anchor[0].ins,
                                   reason="pool queue order")

            def side(rg, maskt, g1, g2, e0t, s3sb):
                mk = rop.tile([RG, 4 * DP], bf16, name="mk", tag="mk")
                nc.sync.dma_start(mk[:], maskt.ap()[rg])
                e0 = rop.tile([RG, D], mybir.dt.float32, name="e0", tag="e0")
                nc.sync.dma_start(e0[:], e0t.ap()[rg])
                acc = rop.tile([RG, D], mybir.dt.float32, name="acc",
                               tag="acc")
                nc.vector.tensor_add(out=acc[:], in0=e0[:],
                                     in1=s3sb[:, rg, :])
                for gqx in (g1, g2):
                    sel = rop.tile([RG, 4 * DP], mybir.dt.float32,
                                   name="sel", tag="sel")
                    nc.vector.tensor_mul(out=sel[:], in0=gqx[:, rg, :],
                                         in1=mk[:])
                    red = rop.tile([RG, D], mybir.dt.float32, name="red",
                                   tag="red")
                    nc.vector.reduce_sum(
                        red[:],
                        sel[:].rearrange("p (r d) -> p d r", r=4)[:, 0:D, :],
                        axis=mybir.AxisListType.X)
                    nc.vector.tensor_add(out=acc[:], in0=acc[:], in1=red[:])
                return acc

            yv = y_out.ap().rearrange("(g p) -> g p", p=RG)
            for rg in range(NRG):
                su = side(rg, mask_u, gq["s1_u"], gq["s2_u"], e0su, s3u_sb)
                si = side(rg, mask_i, gq["s1_i"], gq["s2_i"], e0si, s3i_sb)
                pr = rop.tile([RG, D], mybir.dt.float32, name="pr", tag="pr")
                nc.vector.tensor_mul(out=pr[:], in0=su[:], in1=si[:])
                nc.vector.tensor_scalar_mul(out=pr[:], in0=pr[:],
                                            scalar1=1.0 / 16.0)
                yc = rop.tile([RG, 1], mybir.dt.float32, name="yc", tag="yc")
                nc.vector.reduce_sum(yc[:], pr[:], axis=mybir.AxisListType.X)
                nc.sync.dma_start(yv[rg], yc[:, 0])

    nc.compile()
    return nc


_CACHE = {}
_TRACE = False
_TRACE_DIR = None
_LAST_RES = None


def _schedule_key(p):
    import hashlib
    h = hashlib.sha1()
    for k in ("ui", "iu", "l3a", "l3b"):
        h.update(p[k]["C"].tobytes())
        h.update(np.int64(p[k]["sumC"]).tobytes())
    return h.hexdigest()


def kernel(user_emb, item_emb, edge_vals, edge_u, edge_i, users, items):
    global _LAST_RES
    from concourse.bass_utils import run_bass_kernel_spmd

    user_emb = np.asarray(user_emb, np.float32)
    item_emb = np.asarray(item_emb, np.float32)
    edge_vals = np.asarray(edge_vals, np.float32)
    edge_u = np.asarray(edge_u, np.int64)
    edge_i = np.asarray(edge_i, np.int64)
    users = np.asarray(users, np.int64)
    items = np.asarray(items, np.int64)

    p = _make_plan(user_emb, item_emb, edge_vals, edge_u, edge_i, users,
                   items)
    maps = _build_device_arrays(p)
    key = _schedule_key(p)
    if _CACHE.get("key") != key:
        _CACHE["nc"] = _build_bass(p)
        _CACHE["key"] = key
    nc = _CACHE["nc"]
    res = run_bass_kernel_spmd(nc, maps, core_ids=list(range(NCORES)),
                               trace=_TRACE, tmpdir=_TRACE_DIR)
    _LAST_RES = res
    y = np.concatenate([res.results[c]["y"] for c in range(NCORES)])
    return y.astype(np.float32)


# revision 27
# speedup vs baseline: 1.5238x; 1.0273x over previous
"""CredLightGCN (3-layer LightGCN propagation + batch dot readout) on 8
Trainium2 NeuronCores.

Strategy (all sizes hardcoded for the nn_CredLightGCN problem):
  * The six SpMMs (2 directions x 3 layers) are computed as PE one-hot
    matmuls: for each destination group of 128 rows, PSUM accumulates
    chunks  out[seg,d] += OH[slot,seg]^T @ G[slot,d]  where OH is a
    one-hot (edge-value-weighted) selection matrix generated ON DEVICE by
    the vector engines:  OH[p, f] = (iota[f] == seg[p]) * val[p]  via a
    fused tensor_scalar(is_equal, mult) with per-partition scalars.  Only
    the tiny seg/val streams (4B/edge) come from HBM, not the 32KB/chunk
    dense M tiles.  One-hot generation alternates DVE / Pool to balance
    engine load.  One slot per edge (no dedup).
  * Layer 1 needs no on-device gathers: G streams from host-expanded edge
    tables (the inputs are known on the host).
  * Layer 2 gathers source rows with gpsimd dma_gather (256B rows, int16
    indices, tables split in 25088-row windows).
  * Layer 3 is batch-funneled: only rows reachable from the 16384 query
    pairs are produced (batch positions are the destination rows).
  * Tables are bf16 padded to 128 cols (256B rows) to satisfy dma_gather's
    256B element constraint; PSUM accumulation stays f32.
  * Cores own disjoint destination-row shards; full tables are rebuilt
    between passes with DRAM AllGather collectives.
    Pass order (ui-L1, iu-L1, iu-L2, ui-L2, l3a, l3b) is chosen
    so every AllGather overlaps the pass that follows it; the next
    gather-pass's first gathers carry explicit deps on the previous
    pass's Pool tail so a gather parked on its AllGather wait cannot
    head-of-line block Pool.SEQ.
  * Readout: gather s1/s2 rows as 1KB "quad" rows (4 padded rows per
    descriptor, index = row//4) in ONE dma_gather per (side, table),
    select the right sub-row with a bf16 mask + axis reduce, add the
    layer-0 and layer-3 terms, multiply sides and row-reduce.

Row permutation: items/users are assigned to device rows by a
degree-balanced snake so every destination group has a near-equal edge
count, which makes the chunk schedule uniform across the 8 cores (all
cores run one shared program; per-core data differs).
"""

import numpy as np
import ml_dtypes

NCORES = 8
GSZ = 128         # dst rows per group (PSUM partitions)
CH = 128          # edge slots per chunk (PE contraction K)
D = 64            # embedding dim
DP = 128          # padded bf16 row width (256B)
BF = ml_dtypes.bfloat16

N_IT_REAL, N_US_REAL = 50000, 100000
UNIT = NCORES * GSZ
N_IT = -(-N_IT_REAL // UNIT) * UNIT          # 50176
N_US = -(-N_US_REAL // UNIT) * UNIT          # 100352
GI, GU = N_IT // GSZ, N_US // GSZ
GI_C, GU_C = GI // NCORES, GU // NCORES
QS = 25088
NQ_US, NQ_IT = -(-N_US // QS), -(-N_IT // QS)    # 4, 2
BATCH = 16384
BPC = BATCH // NCORES
G3 = BPC // GSZ
RG = 128
NRG = BPC // RG


# --------------------------------------------------------------------------
# host planning
# --------------------------------------------------------------------------

def _balanced_perm(deg, n_pad, n_groups):
    n_real = len(deg)
    order = np.argsort(-deg, kind="stable")
    order = np.concatenate([order, np.arange(n_real, n_pad)])
    gsz = n_pad // n_groups
    pi = np.empty(n_pad, np.int64)
    for r in range(gsz):
        blk = order[r * n_groups:(r + 1) * n_groups]
        cells = np.arange(n_groups) if r % 2 == 0 else \
            np.arange(n_groups - 1, -1, -1)
        pi[blk] = cells * gsz + r
    return pi


def _build_dir_layout(dst_rows, src_rows, vals, groups_per_core, nq, qsize):
    """One slot per edge, sorted (core, group, q, src) for gather locality."""
    g = dst_rows // GSZ
    seg = (dst_rows % GSZ).astype(np.int32)
    q = src_rows // qsize
    srcl = src_rows % qsize
    core = g // groups_per_core
    gl = g % groups_per_core

    sort_key = np.lexsort((srcl, q, gl, core))
    core_s, gl_s = core[sort_key], gl[sort_key]
    q_s, srcl_s = q[sort_key], srcl[sort_key]
    seg_s, val_s = seg[sort_key], vals[sort_key]
    ck = (core_s * groups_per_core + gl_s) * nq + q_s
    nruns = NCORES * groups_per_core * nq
    run_start = np.searchsorted(ck, np.arange(nruns + 1))
    cnt = (run_start[1:] - run_start[:-1]).reshape(
        NCORES, groups_per_core, nq)
    rank = np.arange(len(ck)) - run_start[ck]

    C = np.maximum(1, -(-cnt.max(axis=0) // CH))
    sumC = int(C.sum(axis=1).max())
    for i in range(groups_per_core):
        C[i, nq - 1] += sumC - C[i].sum()
    qoff = np.zeros((groups_per_core, nq + 1), np.int64)
    qoff[:, 1:] = np.cumsum(C, axis=1)

    nslots = sumC * CH
    slot = (qoff[gl_s, q_s] * CH + rank).astype(np.int64)

    srcs = np.zeros((NCORES, groups_per_core, nslots), np.int32)
    srcs[core_s, gl_s, slot] = (q_s * qsize + srcl_s).astype(np.int32)
    pad = np.ones((NCORES, groups_per_core, nslots), bool)
    pad[core_s, gl_s, slot] = False
    c_of = np.arange(nslots) // CH
    qof_slot = np.zeros((groups_per_core, nslots), np.int64)
    for i in range(groups_per_core):
        qq = np.searchsorted(qoff[i], c_of, side="right") - 1
        qof_slot[i] = np.minimum(qq, nq - 1) * qsize
    srcs = np.where(pad, qof_slot[None, :, :], srcs)

    return dict(C=C, sumC=sumC, qoff=qoff, src=srcs,
                e_core=core_s, e_gl=gl_s, e_slot=slot, e_seg=seg_s,
                e_val=val_s, nq=nq, qsize=qsize,
                groups_per_core=groups_per_core)


def _layout_arrays(lay):
    """seg||val stream [NC, gpc, CH, 2*sumC] bf16 and wrapped idx tables."""
    gpc, sumC = lay["groups_per_core"], lay["sumC"]
    nslots = sumC * CH
    segval = np.zeros((NCORES, gpc, CH, 2 * sumC), np.float32)
    p = lay["e_slot"] % CH
    cx = lay["e_slot"] // CH
    segval[lay["e_core"], lay["e_gl"], p, cx] = \
        lay["e_seg"].astype(np.float32)
    segval[lay["e_core"], lay["e_gl"], p, sumC + cx] = lay["e_val"]
    locidx = (lay["src"] % lay["qsize"]).astype(np.int16)
    w = locidx.reshape(NCORES, gpc, nslots // 16, 16)
    w = np.swapaxes(w, 2, 3)
    idx = np.tile(w, (1, 1, 8, 1))
    return segval, idx


def _expand_E(lay, table_glob):
    gpc, sumC = lay["groups_per_core"], lay["sumC"]
    E = table_glob[lay["src"]]
    E = E.reshape(NCORES, gpc, sumC, CH, D)
    return np.ascontiguousarray(np.swapaxes(E, 2, 3)).astype(BF)


def _make_plan(user_emb, item_emb, edge_vals, edge_u, edge_i, users, items):
    p = {}
    deg_it = np.bincount(edge_i, minlength=N_IT_REAL)
    deg_us = np.bincount(edge_u, minlength=N_US_REAL)
    pi_it = _balanced_perm(deg_it, N_IT, GI)
    pi_us = _balanced_perm(deg_us, N_US, GU)

    t0_us = np.zeros((N_US, D), np.float32)
    t0_us[pi_us[:N_US_REAL]] = user_emb
    t0_it = np.zeros((N_IT, D), np.float32)
    t0_it[pi_it[:N_IT_REAL]] = item_emb
    p["t0_us"], p["t0_it"] = t0_us, t0_it

    dst_it = pi_it[edge_i]
    dst_us = pi_us[edge_u]
    ev = edge_vals.astype(np.float32)
    p["ui"] = _build_dir_layout(dst_it, dst_us, ev, GI_C, NQ_US, QS)
    p["iu"] = _build_dir_layout(dst_us, dst_it, ev, GU_C, NQ_IT, QS)

    def edges_of(ids_batch, by_node_sorted, node_ptr, other_rows, vals):
        cnts = node_ptr[ids_batch + 1] - node_ptr[ids_batch]
        tot = int(cnts.sum())
        pos_rep = np.repeat(np.arange(len(ids_batch)), cnts)
        starts = np.repeat(node_ptr[ids_batch], cnts)
        within = np.arange(tot) - np.repeat(np.cumsum(cnts) - cnts, cnts)
        eidx = by_node_sorted[starts + within]
        return pos_rep.astype(np.int64), other_rows[eidx], vals[eidx]

    o_i = np.argsort(edge_i, kind="stable")
    ptr_i = np.zeros(N_IT_REAL + 1, np.int64)
    ptr_i[1:] = np.cumsum(deg_it)
    o_u = np.argsort(edge_u, kind="stable")
    ptr_u = np.zeros(N_US_REAL + 1, np.int64)
    ptr_u[1:] = np.cumsum(deg_us)

    posA, srcA, valA = edges_of(items, o_i, ptr_i, dst_us, ev)
    posB, srcB, valB = edges_of(users, o_u, ptr_u, dst_it, ev)
    p["l3a"] = _build_dir_layout(posA, srcA, valA, G3, NQ_US, QS)
    p["l3b"] = _build_dir_layout(posB, srcB, valB, G3, NQ_IT, QS)

    p["bu_rows"] = pi_us[users].reshape(NCORES, BPC)
    p["bi_rows"] = pi_it[items].reshape(NCORES, BPC)
    p["e0u_b"] = user_emb[users].reshape(NCORES, BPC, D).astype(np.float32)
    p["e0i_b"] = item_emb[items].reshape(NCORES, BPC, D).astype(np.float32)
    return p


def _build_device_arrays(p):
    maps = [dict() for _ in range(NCORES)]
    sv_ui, idx_ui = _layout_arrays(p["ui"])
    sv_iu, idx_iu = _layout_arrays(p["iu"])
    sv_3a, idx_3a = _layout_arrays(p["l3a"])
    sv_3b, idx_3b = _layout_arrays(p["l3b"])
    E_ui = _expand_E(p["ui"], p["t0_us"])
    E_iu = _expand_E(p["iu"], p["t0_it"])

    def readout_arrays(rows):
        # one batched gather per table: 2048 quad indices, wrapped 16-wide
        quad = (rows // 4).astype(np.int16)             # [NC, BPC]
        r = (rows.reshape(NCORES, NRG, RG) % 4).astype(np.int64)
        w = quad.reshape(NCORES, BPC // 16, 16)
        w = np.swapaxes(w, 1, 2)                        # [NC, 16, BPC//16]
        idxr = np.tile(w, (1, 8, 1))                    # [NC, 128, BPC//16]
        mask = np.zeros((NCORES, NRG, RG, 4 * DP), BF)
        cc = np.arange(NCORES)[:, None, None]
        gg = np.arange(NRG)[None, :, None]
        kk = np.arange(RG)[None, None, :]
        for d in range(D):
            mask[cc, gg, kk, r * DP + d] = 1.0
        return idxr, mask

    idxr_u, mask_u = readout_arrays(p["bu_rows"])
    idxr_i, mask_i = readout_arrays(p["bi_rows"])

    for c in range(NCORES):
        m = maps[c]
        m["sv_ui"], m["idx_ui"], m["E_ui"] = sv_ui[c], idx_ui[c], E_ui[c]
        m["sv_iu"], m["idx_iu"], m["E_iu"] = sv_iu[c], idx_iu[c], E_iu[c]
        m["sv_3a"], m["idx_3a"] = sv_3a[c], idx_3a[c]
        m["sv_3b"], m["idx_3b"] = sv_3b[c], idx_3b[c]
        m["idxr_u"], m["mask_u"] = idxr_u[c], mask_u[c]
        m["idxr_i"], m["mask_i"] = idxr_i[c], mask_i[c]
        m["e0su"] = p["e0u_b"][c].reshape(NRG, RG, D)
        m["e0si"] = p["e0i_b"][c].reshape(NRG, RG, D)
    return maps


# --------------------------------------------------------------------------
# bass program
# --------------------------------------------------------------------------

def _build_bass(p):
    import concourse.bacc as bacc
    import concourse.tile as tile
    import concourse.mybir as mybir

    from concourse.tile import add_dep_helper

    f32, i16, bf16 = mybir.dt.float32, mybir.dt.int16, mybir.dt.bfloat16
    EQ, MUL = mybir.AluOpType.is_equal, mybir.AluOpType.mult
    nc = bacc.Bacc("TRN2", target_bir_lowering=False, debug=False,
                   num_devices=NCORES)

    def din(name, shape, dt=bf16):
        return nc.dram_tensor(name, list(shape), dt, kind="ExternalInput")

    lays = {}
    for nm, lay, with_e in [("ui", p["ui"], True), ("iu", p["iu"], True),
                            ("3a", p["l3a"], False), ("3b", p["l3b"], False)]:
        gpc, sumC = lay["groups_per_core"], lay["sumC"]
        t = dict(lay=lay, gpc=gpc, sumC=sumC)
        t["sv"] = din(f"sv_{nm}", [gpc, CH, 2 * sumC], f32)
        t["idx"] = din(f"idx_{nm}", [gpc, CH, sumC * CH // 16], i16)
        if with_e:
            t["E"] = din(f"E_{nm}", [gpc, CH, sumC, D])
        lays[nm] = t
    idxr_u = din("idxr_u", [128, BPC // 16], i16)
    idxr_i = din("idxr_i", [128, BPC // 16], i16)
    mask_u = din("mask_u", [NRG, RG, 4 * DP])
    mask_i = din("mask_i", [NRG, RG, 4 * DP])
    e0su = din("e0su", [NRG, RG, D], f32)
    e0si = din("e0si", [NRG, RG, D], f32)
    y_out = nc.dram_tensor("y", [BPC], f32, kind="ExternalOutput")

    reps = [list(range(NCORES))]

    with tile.TileContext(nc) as tc:
        with (
            tc.tile_pool(name="svt", bufs=3) as svp,
            tc.tile_pool(name="oht", bufs=8) as ohp,
            tc.tile_pool(name="gt", bufs=3) as gtp,
            tc.tile_pool(name="ixt", bufs=4) as ixp,
            tc.tile_pool(name="ps", bufs=8, space="PSUM") as psp,
            tc.tile_pool(name="ev", bufs=4) as evp,
            tc.tile_pool(name="ro", bufs=4) as rop,
            tc.tile_pool(name="roq", bufs=1) as roqp,
            tc.tile_pool(name="s3", bufs=1) as s3p,
            tc.tile_pool(name="cst", bufs=1) as cstp,
            tc.tile_pool(name="dram", bufs=1, space="DRAM") as drp,
        ):
            sh = {
                "s1_i": drp.tile([GI_C * GSZ, DP], bf16, name="s1_i_sh"),
                "s1_u": drp.tile([GU_C * GSZ, DP], bf16, name="s1_u_sh"),
                "s2_i": drp.tile([GI_C * GSZ, DP], bf16, name="s2_i_sh"),
                "s2_u": drp.tile([GU_C * GSZ, DP], bf16, name="s2_u_sh"),
            }
            fl = {
                "s1_i": drp.tile([N_IT, DP], bf16, name="s1_i_f"),
                "s1_u": drp.tile([N_US, DP], bf16, name="s1_u_f"),
                "s2_i": drp.tile([N_IT, DP], bf16, name="s2_i_f"),
                "s2_u": drp.tile([N_US, DP], bf16, name="s2_u_f"),
            }
            s3i_sb = s3p.tile([128, G3, D], mybir.dt.float32, name="s3i_sb")
            s3u_sb = s3p.tile([128, G3, D], mybir.dt.float32, name="s3u_sb")

            iota_t = cstp.tile([128, 128], bf16, name="iota_t")
            nc.gpsimd.iota(iota_t[:], pattern=[[1, 128]], base=0,
                           channel_multiplier=0,
                           allow_small_or_imprecise_dtypes=True)

            ev_tiles = []
            for j in range(4):
                t_ = evp.tile([GSZ, DP], bf16, name=f"evst{j}", tag=f"evst{j}")
                nc.vector.memset(t_[:], 0.0)
                ev_tiles.append(t_)

            anchor = [None]   # last Pool instruction of the previous pass

            def run_pass(t, src_tab, n_src, nq, dst_shard, dst_s3):
                lay, gpc, sumC = t["lay"], t["gpc"], t["sumC"]
                C, qoff = lay["C"], lay["qoff"]
                stream = src_tab is None
                prev_anchor, last_pool = anchor[0], None
                for g in range(gpc):
                    sv = svp.tile([CH, 2 * sumC], mybir.dt.float32,
                                  name="sv", tag="sv")
                    nc.sync.dma_start(sv[:], t["sv"].ap()[g])
                    if stream:
                        gt = gtp.tile([CH, sumC, D], bf16, name="gts",
                                      tag="gts")
                        nc.sync.dma_start(gt[:], t["E"].ap()[g])
                        rhs = lambda c: gt[:, c, :]
                    else:
                        gt = gtp.tile([CH, sumC, DP], bf16, name="gtg",
                                      tag="gtg")
                        ixt = ixp.tile([CH, sumC * CH // 16], i16,
                                       name="ixt", tag="ixt")
                        nc.sync.dma_start(ixt[:], t["idx"].ap()[g])
                        for q in range(nq):
                            cq, off = int(C[g, q]), int(qoff[g, q])
                            if cq == 0:
                                continue
                            qlo = q * QS
                            qhi = min((q + 1) * QS, n_src)
                            gi = nc.gpsimd.dma_gather(
                                gt[:, off:off + cq, :],
                                src_tab.opt()[qlo:qhi],
                                ixt[:, off * 8:(off + cq) * 8],
                                cq * CH, cq * CH, DP,
                                single_packet=False,
                            )
                            # Pin early gathers behind the previous pass's
                            # Pool tail: a hoisted gather parks on Pool.SEQ
                            # waiting for the AllGather and head-of-line
                            # blocks every later Pool instruction.
                            if g < 4 and prev_anchor is not None:
                                add_dep_helper(gi.ins, prev_anchor.ins,
                                               reason="pool queue order")
                            last_pool = gi
                        rhs = lambda c: gt[:, c, 0:D]
                    ps = psp.tile([GSZ, D], mybir.dt.float32, name="ps",
                                  tag="ps", space="PSUM")
                    for cx in range(sumC):
                        oh = ohp.tile([CH, GSZ], bf16, name="oh", tag="oh")
                        # Pool takes a third of the stream-pass one-hots
                        # (it is ~3x slower per op and busy with gathers in
                        # gather passes).
                        eng = nc.gpsimd if (stream and cx % 3 == 2) \
                            else nc.vector
                        ts_i = eng.tensor_scalar(
                            out=oh[:], in0=iota_t[:],
                            scalar1=sv[:, cx:cx + 1],
                            scalar2=sv[:, sumC + cx:sumC + cx + 1],
                            op0=EQ, op1=MUL)
                        if eng is nc.gpsimd:
                            last_pool = ts_i
                        nc.tensor.matmul(ps[:], lhsT=oh[:],
                                         rhs=rhs(cx), start=(cx == 0),
                                         stop=(cx == sumC - 1))
                    if dst_s3 is None:
                        ev = ev_tiles[g % 4]
                        nc.scalar.copy(ev[:, 0:D], ps[:])
                        nc.sync.dma_start(
                            dst_shard.opt()[g * GSZ:(g + 1) * GSZ, :], ev[:])
                    else:
                        nc.scalar.copy(dst_s3[:, g, :], ps[:])
                if last_pool is not None:
                    anchor[0] = last_pool

            def ag(shard, full):
                nc.gpsimd.collective_compute(
                    "AllGather", mybir.AluOpType.bypass, replica_groups=reps,
                    ins=[shard.opt()], outs=[full.opt()])

            # Every AllGather overlaps the immediately following pass:
            #   AG(s1_i) under iu-L1, AG(s1_u) under iu-L2 (consumes s1_i),
            #   AG(s2_u) under ui-L2 (consumes s1_u), AG(s2_i) under l3a
            #   (consumes s2_u); l3b consumes s2_i.
            run_pass(lays["ui"], None, 0, 0, sh["s1_i"], None)
            ag(sh["s1_i"], fl["s1_i"])
            run_pass(lays["iu"], None, 0, 0, sh["s1_u"], None)
            ag(sh["s1_u"], fl["s1_u"])
            run_pass(lays["iu"], fl["s1_i"], N_IT, NQ_IT, sh["s2_u"], None)
            ag(sh["s2_u"], fl["s2_u"])
            run_pass(lays["ui"], fl["s1_u"], N_US, NQ_US, sh["s2_i"], None)
            ag(sh["s2_i"], fl["s2_i"])
            run_pass(lays["3a"], fl["s2_u"], N_US, NQ_US, None, s3i_sb)
            run_pass(lays["3b"], fl["s2_i"], N_IT, NQ_IT, None, s3u_sb)

            qv = {k: fl[k].opt().rearrange("(n r) d -> n (r d)", r=4)
                  for k in fl}

            # batched readout quad gathers: one call per (side, table)
            gq = {}
            for nm, idxr in (("s1_u", idxr_u), ("s2_u", idxr_u),
                             ("s1_i", idxr_i), ("s2_i", idxr_i)):
                ixr = roqp.tile([128, BPC // 16], i16, name=f"ixr_{nm}")
                nc.sync.dma_start(ixr[:], idxr.ap())
                gq[nm] = roqp.tile([RG, NRG, 4 * DP], bf16, name=f"gq_{nm}")
                gi = nc.gpsimd.dma_gather(gq[nm][:], qv[nm], ixr[:], BPC,
                                          BPC, 4 * DP, single_packet=False)
                if anchor[0] is not None:
                    add_dep_helper(gi.ins, anchor[0].ins,
                                   reason="pool queue order")

            def side(rg, maskt, g1, g2, e0t, s3sb):
                mk = rop.tile([RG, 4 * DP], bf16, name="mk", tag="mk")
                nc.sync.dma_start(mk[:], maskt.ap()[rg])
                e0 = rop.tile([RG, D], mybir.dt.float32, name="e0", tag="e0")
                nc.sync.dma_start(e0[:], e0t.ap()[rg])
                acc = rop.tile([RG, D], mybir.dt.float32, name="acc",
                               tag="acc")
                nc.vector.tensor_add(out=acc[:], in0=e0[:],
                                     in1=s3sb[:, rg, :])
                for gqx in (g1, g2):
                    sel = rop.tile([RG, 4 * DP], mybir.dt.float32,
                                   name="sel", tag="sel")
                    nc.vector.tensor_mul(out=sel[:], in0=gqx[:, rg, :],
                                         in1=mk[:])
                    red = rop.tile([RG, D], mybir.dt.float32, name="red",
                                   tag="red")
                    nc.vector.reduce_sum(
                        red[:],
                        sel[:].rearrange("p (r d) -> p d r", r=4)[:, 0:D, :],
                        axis=mybir.AxisListType.X)
                    nc.vector.tensor_add(out=acc[:], in0=acc[:], in1=red[:])
                return acc

            yv = y_out.ap().rearrange("(g p) -> g p", p=RG)
            for rg in range(NRG):
                su = side(rg, mask_u, gq["s1_u"], gq["s2_u"], e0su, s3u_sb)
                si = side(rg, mask_i, gq["s1_i"], gq["s2_i"], e0si, s3i_sb)
                pr = rop.tile([RG, D], mybir.dt.float32, name="pr", tag="pr")
                nc.vector.tensor_mul(out=pr[:], in0=su[:], in1=si[:])
                nc.vector.tensor_scalar_mul(out=pr[:], in0=pr[:],
                                            scalar1=1.0 / 16.0)
                yc = rop.tile([RG, 1], mybir.dt.float32, name="yc", tag="yc")
                nc.vector.reduce_sum(yc[:], pr[:], axis=mybir.AxisListType.X)
                nc.sync.dma_start(yv[rg], yc[:, 0])

    nc.compile()
    return nc


_CACHE = {}
_TRACE = False
_TRACE_DIR = None
_LAST_RES = None


def _schedule_key(p):
    import hashlib
    h = hashlib.sha1()
    for k in ("ui", "iu", "l3a", "l3b"):
        h.update(p[k]["C"].tobytes())
        h.update(np.int64(p[k]["sumC"]).tobytes())
    return h.hexdigest()


def kernel(user_emb, item_emb, edge_vals, edge_u, edge_i, users, items):
    global _LAST_RES
    from concourse.bass_utils import run_bass_kernel_spmd

    user_emb = np.asarray(user_emb, np.float32)
    item_emb = np.asarray(item_emb, np.float32)
    edge_vals = np.asarray(edge_vals, np.float32)
    edge_u = np.asarray(edge_u, np.int64)
    edge_i = np.asarray(edge_i, np.int64)
    users = np.asarray(users, np.int64)
    items = np.asarray(items, np.int64)

    p = _make_plan(user_emb, item_emb, edge_vals, edge_u, edge_i, users,
                   items)
    maps = _build_device_arrays(p)
    key = _schedule_key(p)
    if _CACHE.get("key") != key:
        _CACHE["nc"] = _build_bass(p)
        _CACHE["key"] = key
    nc = _CACHE["nc"]
    res = run_bass_kernel_spmd(nc, maps, core_ids=list(range(NCORES)),
                               trace=_TRACE, tmpdir=_TRACE_DIR)
    _LAST_RES = res
    y = np.concatenate([res.results[c]["y"] for c in range(NCORES)])
    return y.astype(np.float32)
